# revision 7
# baseline (speedup 1.0000x reference)
"""Trainium2 Bass kernel for the per-channel CDF-flow MLP (position-sharded).

Math (per channel c, elementwise over N positions):
    u0 = W0 x + b0          v0 = u0 + T0*tanh(u0)     (W* = softplus(m*), T* = tanh(f*))
    u1 = W1 v0 + b1         v1 = u1 + T1*tanh(u1)
    u2 = W2 v1 + b2         v2 = u2 + T2*tanh(u2)
    out = W3 v2 + b3

Sharding: positions (65536) split across 8 cores, 8192 each; every core
holds all 256 channels. The bottleneck engine is ACT (9 tanh per
channel-position) and its cost is free-dim-proportional and
partition-count-independent, so channels are packed 42-per-tile
(126 of 128 partitions useful vs 96 in a 32-channel layout):
  - 6 "patterns" of 42 channels -> [126, F] tiles, rows j-major (42j + c)
  - 4-channel tail repacked as 32 (channel, pos-block) units -> [96, F]
Per unit (pattern x 1024-col chunk):
  xE   : rows 0..41 = x (DMA), rows 42..125 = x replicated (SBUF->SBUF DMA)
  t0   = tanh(W0*xE + c0)                       (ACT, per-partition scale+bias)
  y1   = A1t.T @ t0 + A1x-cols.T @ x            (PE, 2 lhsT; A1t = W1 diag(T0),
                                                 A1x = W1 @ W0; PSUM)
  t1   = tanh(y1 + c1)                          (ACT)
  z2   = T1*t1 + y1                             (DVE scalar_tensor_tensor)
  y2   = W2.T-blockdiag @ z2                    (PE)
  t2   = tanh(y2 + c2)                          (ACT)
  pack += A3z.T @ z2 + A3t.T @ t2               (PE; A3z = W3 @ W2, A3t = W3
                                                 diag(T2); 3 patterns -> 126 rows)
  osb  = pack + c3; DMA out                     (DVE)
with c0 = b0, c1 = W1 b0 + b1, c2 = W2 c1 + b2, c3 = W3 c2 + b3.

Software pipeline (stages phased so ACT streams back-to-back):
iteration i emits  A(i+4) dma, B(i+3) t0, C(i+2) y1, D(i+2) t1,
                   E(i+1) z2, F(i+1) y2, G(i+1) t2, H(i) pack.
PSUM budget: y1 pool bufs=2 (4 banks) + y2 bufs=1 (2) + pack bufs=1 (2) = 8.
"""

import os
from contextlib import ExitStack, nullcontext

import numpy as np

import concourse.bacc as bacc
import concourse.bass as bass
import concourse.tile as tile
from concourse import mybir
from concourse.bass_utils import run_bass_kernel_spmd

F32 = mybir.dt.float32
F32R = mybir.dt.float32r

CH = 256
NPOS = 65536
NCORES = 8
POSC = NPOS // NCORES       # 8192 positions per core
CHP = 42                    # channels per pattern tile
NPAT = 6                    # full patterns (252 channels)
R = 3 * CHP                 # 126 rows per pattern tile
TCH = 4                     # tail channels (252..255)
TB = 8                      # tail position blocks of F
TU = TCH * TB               # 32 tail units
TR = 3 * TU                 # 96 tail rows
F = 1024                    # free-dim chunk (PSUM tile = 2 banks of f32)
MMN = 512                   # matmul free-dim slice (one PSUM bank)
NCHUNK = POSC // F          # 8
NU = NCHUNK * NPAT          # 48 pattern units; unit NU is the tail

LAST_RESULTS = None         # test.py introspects this


def _softplus(x):
    return np.logaddexp(0.0, x.astype(np.float64))


def _host_params(m0, m1, m2, m3, b0, b1, b2, b3, f0, f1, f2):
    """Fold weights/biases/gates into the device parameterization (float64)."""
    W0 = _softplus(m0)[:, :, 0]
    W1 = _softplus(m1)
    W2 = _softplus(m2)
    W3 = _softplus(m3)[:, 0, :]
    b0_ = b0.astype(np.float64)[:, :, 0]
    b1_ = b1.astype(np.float64)[:, :, 0]
    b2_ = b2.astype(np.float64)[:, :, 0]
    b3_ = b3.astype(np.float64)[:, 0, 0]
    T0 = np.tanh(f0.astype(np.float64))[:, :, 0]
    T1 = np.tanh(f1.astype(np.float64))[:, :, 0]
    T2 = np.tanh(f2.astype(np.float64))[:, :, 0]
    c0 = b0_
    c1 = np.einsum("cjk,ck->cj", W1, b0_) + b1_
    c2 = np.einsum("cjk,ck->cj", W2, c1) + b2_
    c3 = np.einsum("ck,ck->c", W3, c2) + b3_
    A1x = np.einsum("cjk,ck->cj", W1, W0)      # W1 @ W0
    A1t = W1 * T0[:, None, :]                  # W1 diag(T0)
    A3z = np.einsum("cm,cmk->ck", W3, W2)      # W3 @ W2
    A3t = W3 * T2                              # W3 diag(T2)
    return dict(A1x=A1x, A1t=A1t, W2=W2, A3z=A3z, A3t=A3t, W0=W0,
                c0=c0, c1=c1, c2=c2, c3=c3, T1=T1)


def _device_arrays(p):
    """Shared (core-independent) device arrays from host params `p`."""
    f32 = np.float32
    arrs = {}
    c = np.arange(CHP)
    for P in range(NPAT):
        sl = slice(CHP * P, CHP * P + CHP)
        A1x, A1t, W2 = p["A1x"][sl], p["A1t"][sl], p["W2"][sl]
        A3z, A3t = p["A3z"][sl], p["A3t"][sl]
        l1t = np.zeros((R, R), f32)
        l2 = np.zeros((R, R), f32)
        d1 = np.zeros((CHP, R), f32)
        g = P % 3
        l3z = np.zeros((R, R), f32)
        l3t = np.zeros((R, R), f32)
        for j in range(3):
            d1[c, CHP * j + c] = A1x[:, j]
            for k in range(3):
                l1t[CHP * k + c, CHP * j + c] = A1t[:, j, k]
                l2[CHP * k + c, CHP * j + c] = W2[:, j, k]
        for k in range(3):
            l3z[CHP * k + c, CHP * g + c] = A3z[:, k]
            l3t[CHP * k + c, CHP * g + c] = A3t[:, k]
        vec = lambda t: np.concatenate(
            [t[:, j] for j in range(3)]).astype(f32).reshape(R, 1)
        arrs[f"l1t{P}"] = l1t
        arrs[f"d1_{P}"] = d1
        arrs[f"l2_{P}"] = l2
        arrs[f"l3z{P}"] = l3z
        arrs[f"l3t{P}"] = l3t
        arrs[f"W0v{P}"] = vec(p["W0"][sl])
        arrs[f"c0v{P}"] = vec(p["c0"][sl])
        arrs[f"c1v{P}"] = vec(p["c1"][sl])
        arrs[f"c2v{P}"] = vec(p["c2"][sl])
        arrs[f"T1v{P}"] = vec(p["T1"][sl])
    arrs["c3vA"] = p["c3"][0:126].astype(f32).reshape(126, 1)
    arrs["c3vB"] = p["c3"][126:252].astype(f32).reshape(126, 1)
    # tail: channels 252..255, unit u = 8c + b, rows 32j + u
    u = np.arange(TU)
    chu = 252 + u // TB
    l1tT = np.zeros((TR, TR), f32)
    l2T = np.zeros((TR, TR), f32)
    d1T = np.zeros((TU, TR), f32)
    l3zT = np.zeros((TR, TU), f32)
    l3tT = np.zeros((TR, TU), f32)
    for j in range(3):
        d1T[u, TU * j + u] = p["A1x"][chu, j]
        for k in range(3):
            l1tT[TU * k + u, TU * j + u] = p["A1t"][chu, j, k]
            l2T[TU * k + u, TU * j + u] = p["W2"][chu, j, k]
    for k in range(3):
        l3zT[TU * k + u, u] = p["A3z"][chu, k]
        l3tT[TU * k + u, u] = p["A3t"][chu, k]
    vecT = lambda t: np.concatenate(
        [t[chu, j] for j in range(3)]).astype(f32).reshape(TR, 1)
    arrs.update(l1tT=l1tT, d1T=d1T, l2T=l2T, l3zT=l3zT, l3tT=l3tT,
                W0vT=vecT(p["W0"]), c0vT=vecT(p["c0"]), c1vT=vecT(p["c1"]),
                c2vT=vecT(p["c2"]), T1vT=vecT(p["T1"]),
                c3vT=p["c3"][chu].astype(f32).reshape(TU, 1))
    return arrs


# (name, shape, dtype) of every device parameter
def _param_specs():
    specs = []
    for P in range(NPAT):
        specs += [(f"l1t{P}", [R, R], F32R), (f"d1_{P}", [CHP, R], F32R),
                  (f"l2_{P}", [R, R], F32R), (f"l3z{P}", [R, R], F32R),
                  (f"l3t{P}", [R, R], F32R)]
        specs += [(f"{v}{P}", [R, 1], F32)
                  for v in ("W0v", "c0v", "c1v", "c2v", "T1v")]
    specs += [("c3vA", [126, 1], F32), ("c3vB", [126, 1], F32)]
    specs += [("l1tT", [TR, TR], F32R), ("d1T", [TU, TR], F32R),
              ("l2T", [TR, TR], F32R), ("l3zT", [TR, TU], F32R),
              ("l3tT", [TR, TU], F32R)]
    specs += [(f"{v}T", [TR, 1], F32)
              for v in ("W0v", "c0v", "c1v", "c2v", "T1v")]
    specs += [("c3vT", [TU, 1], F32)]
    return specs


def build_nc(repeat=1):
    """Build the per-core Bass program (SPMD: same program + weights,
    per-core x/o)."""
    nc = bacc.Bacc("TRN2", target_bir_lowering=False, debug=False)
    x_d = nc.declare_dram_parameter("x", [CH, POSC], F32R, isOutput=False)
    o_d = nc.declare_dram_parameter("o", [CH, POSC], F32, isOutput=True)
    pd = {}
    for name, shape, dt in _param_specs():
        pd[name] = (nc.declare_dram_parameter(name, shape, dt, isOutput=False), dt)

    Tanh = mybir.ActivationFunctionType.Tanh
    mult = mybir.AluOpType.mult
    add = mybir.AluOpType.add

    with tile.TileContext(nc) as tc, ExitStack() as ctx:
        singles = ctx.enter_context(tc.tile_pool(name="singles", bufs=1))
        xin = ctx.enter_context(tc.tile_pool(name="xin", bufs=5))
        t0p = ctx.enter_context(tc.tile_pool(name="t0p", bufs=3))
        t1p = ctx.enter_context(tc.tile_pool(name="t1p", bufs=2))
        z2p = ctx.enter_context(tc.tile_pool(name="z2p", bufs=2))
        t2p = ctx.enter_context(tc.tile_pool(name="t2p", bufs=2))
        osbp = ctx.enter_context(tc.tile_pool(name="osbp", bufs=3))
        psy1 = ctx.enter_context(tc.tile_pool(name="psy1", bufs=2, space="PSUM"))
        psy2 = ctx.enter_context(tc.tile_pool(name="psy2", bufs=1, space="PSUM"))
        psp = ctx.enter_context(tc.tile_pool(name="psp", bufs=1, space="PSUM"))

        w = {}
        for name, (d, dt) in pd.items():
            t = singles.tile(list(d.shape), dt, tag=name, name=name)
            nc.sync.dma_start(out=t[:], in_=d[:])
            w[name] = t

        xa = x_d[:]
        oa = o_d[:]

        def x_src_ap(u):
            """DRAM AP for unit u's x block replicated 3x (j-major rows):
            row n*j + i reads the same DRAM row for all j."""
            if u == NU:
                return bass.AP(tensor=xa.tensor, offset=xa.offset + 252 * POSC,
                               ap=[[0, 3], [F, TU], [1, F]])
            t, P = divmod(u, NPAT)
            return bass.AP(tensor=xa.tensor,
                           offset=xa.offset + CHP * P * POSC + F * t,
                           ap=[[0, 3], [POSC, CHP], [1, F]])



        def o_dst_ap(t, g):
            return bass.AP(tensor=oa.tensor,
                           offset=oa.offset + 126 * g * POSC + F * t,
                           ap=[[POSC, 126], [1, F]])

        def o_tail_ap():
            return bass.AP(tensor=oa.tensor, offset=oa.offset + 252 * POSC,
                           ap=[[POSC, TCH], [F, TB], [1, F]])

        def udims(u):
            """(rows, xrows, suffix) for unit u."""
            if u == NU:
                return TR, TU, "T"
            return R, CHP, str(u % NPAT)

        loop_cm = tc.For_i(0, repeat, 1) if repeat > 1 else nullcontext()
        with loop_cm:
            stA, stB, stC, stD, stE, stG = {}, {}, {}, {}, {}, {}
            packs = {}
            SS = [slice(s * MMN, (s + 1) * MMN) for s in range(F // MMN)]

            def emit_A(u):
                rows, xr, _ = udims(u)
                xE = xin.tile([R, F], F32R, tag="xE", name="xE")
                nc.sync.dma_start(out=xE[0:rows, 0:F], in_=x_src_ap(u))
                stA[u] = xE

            def emit_B(u):
                rows, xr, sx = udims(u)
                xE = stA[u]
                t0 = t0p.tile([R, F], F32R, tag="t0", name="t0")
                nc.scalar.activation(t0[0:rows, :], xE[0:rows, :], Tanh,
                                     bias=w["c0v" + sx][:], scale=w["W0v" + sx][:])
                stB[u] = t0

            def emit_C(u):
                rows, xr, sx = udims(u)
                xE = stA.pop(u)
                t0 = stB.pop(u)
                y1 = psy1.tile([R, F], F32, tag="y1", name="y1")
                for ss in SS:
                    nc.tensor.matmul(y1[0:rows, ss], w["l1t" + sx][:],
                                     t0[0:rows, ss], start=True, stop=False)
                for ss in SS:
                    nc.tensor.matmul(y1[0:rows, ss],
                                     w[("d1_" + sx) if u != NU else "d1T"][:],
                                     xE[0:xr, ss], start=False, stop=True)
                stC[u] = y1

            def emit_D(u):
                rows, _, sx = udims(u)
                y1 = stC[u]
                t1 = t1p.tile([R, F], F32R, tag="t1", name="t1")
                nc.scalar.activation(t1[0:rows, :], y1[0:rows, :], Tanh,
                                     bias=w["c1v" + sx][:])
                stD[u] = t1

            def emit_E(u):
                rows, _, sx = udims(u)
                y1 = stC.pop(u)
                t1 = stD.pop(u)
                z2 = z2p.tile([R, F], F32R, tag="z2", name="z2")
                nc.vector.scalar_tensor_tensor(
                    z2[0:rows, :], t1[0:rows, :], w["T1v" + sx][:],
                    y1[0:rows, :], op0=mult, op1=add)
                stE[u] = z2

            def emit_F(u):
                rows, _, sx = udims(u)
                z2 = stE[u]
                y2 = psy2.tile([R, F], F32, tag="y2", name="y2")
                for ss in SS:
                    nc.tensor.matmul(y2[0:rows, ss],
                                     w[("l2_" + sx) if u != NU else "l2T"][:],
                                     z2[0:rows, ss], start=True, stop=True)
                stG[u] = y2

            def emit_G(u):
                rows, _, sx = udims(u)
                y2 = stG.pop(u)
                t2 = t2p.tile([R, F], F32R, tag="t2")
                nc.scalar.activation(t2[0:rows, :], y2[0:rows, :], Tanh,
                                     bias=w["c2v" + sx][:])
                stD[("t2", u)] = t2

            def emit_H(u):
                rows, _, sx = udims(u)
                z2 = stE.pop(u)
                t2 = stD.pop(("t2", u))
                if u == NU:
                    pk = psp.tile([R, F], F32, tag="pack", name="pack")
                    for ss in SS:
                        nc.tensor.matmul(pk[0:TU, ss], w["l3zT"][:],
                                         z2[0:TR, ss], start=True, stop=False)
                    for ss in SS:
                        nc.tensor.matmul(pk[0:TU, ss], w["l3tT"][:],
                                         t2[0:TR, ss], start=False, stop=True)
                    osb = osbp.tile([R, F], F32, tag="osb", name="osb")
                    nc.vector.tensor_scalar_add(osb[0:TU, :], pk[0:TU, :],
                                                w["c3vT"][:])
                    nc.gpsimd.dma_start(out=o_tail_ap(), in_=osb[0:TU, 0:F])
                    return
                t, P = divmod(u, NPAT)
                g = P // 3
                first, last = P % 3 == 0, P % 3 == 2
                if first:
                    packs[(t, g)] = psp.tile([R, F], F32, tag="pack", name="pack")
                pk = packs[(t, g)]
                for ss in SS:
                    nc.tensor.matmul(pk[:, ss], w["l3z" + sx][:], z2[:, ss],
                                     start=first, stop=False)
                for ss in SS:
                    nc.tensor.matmul(pk[:, ss], w["l3t" + sx][:], t2[:, ss],
                                     start=False, stop=last)
                if last:
                    pk = packs.pop((t, g))
                    osb = osbp.tile([R, F], F32, tag="osb", name="osb")
                    nc.vector.tensor_scalar_add(
                        osb[:], pk[:], w["c3vA" if g == 0 else "c3vB"][:])
                    nc.gpsimd.dma_start(out=o_dst_ap(t, g), in_=osb[:])

            # prologue
            for u in (0, 1, 2, 3):
                emit_A(u)
            for u in (0, 1, 2):
                emit_B(u)
            emit_C(0)
            emit_D(0)
            emit_C(1)
            emit_D(1)
            emit_E(0)
            emit_F(0)
            emit_G(0)
            # steady state: iteration i emits A(i+4) B(i+3) C/D(i+2)
            # E/F/G(i+1) H(i)
            for i in range(NU + 1):
                if i + 4 <= NU:
                    emit_A(i + 4)
                if i + 3 <= NU:
                    emit_B(i + 3)
                if i + 2 <= NU:
                    emit_C(i + 2)
                    emit_D(i + 2)
                if i + 1 <= NU:
                    emit_E(i + 1)
                    emit_F(i + 1)
                    emit_G(i + 1)
                emit_H(i)

    nc.finalize()
    return nc


def kernel(inputs, m0, m1, m2, m3, b0, b1, b2, b3, f0, f1, f2, stop_gradient):
    global LAST_RESULTS
    del stop_gradient  # False in setup_inputs; forward math identical anyway
    in_maps = make_in_maps(inputs, m0, m1, m2, m3, b0, b1, b2, b3, f0, f1, f2)

    nc = build_nc()
    res = run_bass_kernel_spmd(
        nc, in_maps, list(range(NCORES)),
        trace=bool(os.environ.get("BASS_TRACE")))
    LAST_RESULTS = res
    out = np.empty((CH, NPOS), dtype=np.float32)
    for g in range(NCORES):
        out[:, g * POSC:(g + 1) * POSC] = res.results[g]["o"]
    return out.reshape(CH, 1, NPOS)


def measure_exec_ns(in_maps, r1=8, r2=1032, n_wall=3):
    """Device-exec-time proxy: wall-clock delta between repeat=r2 and
    repeat=r1 kernels (upload/dispatch overheads cancel in the delta)."""
    import time as _time
    walls = {}
    for rep in (r1, r2):
        nc = build_nc(repeat=rep)
        best = None
        for it in range(n_wall):
            t0 = _time.perf_counter()
            run_bass_kernel_spmd(nc, in_maps, list(range(NCORES)))
            dt = _time.perf_counter() - t0
            if it > 0:  # first call pays compile
                best = dt if best is None else min(best, dt)
        walls[rep] = best
    return (walls[r2] - walls[r1]) / (r2 - r1) * 1e9, walls


def make_in_maps(inputs, m0, m1, m2, m3, b0, b1, b2, b3, f0, f1, f2):
    inputs = np.asarray(inputs, dtype=np.float32)
    params = _host_params(
        *(np.asarray(a) for a in (m0, m1, m2, m3, b0, b1, b2, b3, f0, f1, f2)))
    arrs = _device_arrays(params)
    x = inputs.reshape(CH, NPOS)
    in_maps = []
    for g in range(NCORES):
        im = {"x": np.ascontiguousarray(x[:, g * POSC:(g + 1) * POSC])}
        im.update(arrs)
        in_maps.append(im)
    return in_maps


# revision 12
# speedup vs baseline: 2.7538x; 2.7538x over previous
"""Trainium2 Bass kernel for the per-channel CDF-flow MLP (position-sharded).

Math (per channel c, elementwise over N positions):
    u0 = W0 x + b0          v0 = u0 + T0*tanh(u0)     (W* = softplus(m*), T* = tanh(f*))
    u1 = W1 v0 + b1         v1 = u1 + T1*tanh(u1)
    u2 = W2 v1 + b2         v2 = u2 + T2*tanh(u2)
    out = W3 v2 + b3

Sharding: positions (65536) split across 8 cores, 8192 each; every core
holds all 256 channels. The bottleneck engine is ACT (9 tanh per
channel-position); ACT cost is free-dim-proportional and partition-count-
independent, so channels are packed 42-per-tile (126/128 partitions
useful vs 96 in a 32-channel layout):
  - 6 "patterns" of 42 channels -> [126, F] tiles, rows j-major (42j + c)
  - 4-channel tail repacked as 32 (channel, pos-block) units -> [96, F]

Data staging (all resident in SBUF, no per-unit DMA):
  xc[G]  [126, 8192] bf16  x for group G (3 patterns), HBM load with 32 KB
                           descriptors + f32->bf16 SWDGE cast, 1/group/rep
  xE[h,P][126, 4096] bf16  x replicated 3x j-major; 3 big SBUF->SBUF copies
                           from xc (no HBM small-descriptor penalty)
  reloads for the next repeat are emitted mid-body right after the last
  reader, so they overlap compute across the For_i boundary.

Per unit (pattern x 1024-col chunk):
  t0   = tanh(W0*xE + c0)                  (ACT, per-partition scale+bias)
  y1   = A1t.T @ t0 + A1x-cols.T @ x       (PE; A1t = W1 diag(T0), A1x = W1@W0)
  t1   = tanh(y1 + c1)                     (ACT)
  z2   = T1*t1 + y1                        (DVE scalar_tensor_tensor)
  y2   = W2-blockdiag.T @ z2               (PE)
  t2   = tanh(y2 + c2)                     (ACT)
  pack += A3z.T @ z2 + A3t.T @ t2          (PE; 3 patterns -> 126 rows)
  osb[:, t-slice] = pack + c3              (DVE; [126, 2048] out batches)
with c0 = b0, c1 = W1 b0 + b1, c2 = W2 c1 + b2, c3 = W3 c2 + b3.

Software pipeline (stages phased so ACT streams back-to-back):
iteration i emits  B(i+3) t0, C(i+2) y1, D(i+2) t1, E(i+1) z2,
                   F(i+1) y2, G(i+1) t2, H(i) pack.
PSUM: y1 bufs=2 (4 banks) + y2 bufs=1 (2) + pack bufs=1 (2) = 8 banks.
Only the x -> tanh / x -> A1x paths see bf16 (~1e-3 final rel err);
all other matmul operands are f32r.
"""

import os
from contextlib import ExitStack, nullcontext

import ml_dtypes
import numpy as np

import concourse.bacc as bacc
import concourse.bass as bass
import concourse.tile as tile
from concourse import mybir
from concourse.bass_utils import run_bass_kernel_spmd

F32 = mybir.dt.float32
F32R = mybir.dt.float32r
BF16 = mybir.dt.bfloat16

CH = 256
NPOS = 65536
NCORES = 8
POSC = NPOS // NCORES       # 8192 positions per core
CHP = 42                    # channels per pattern tile
NPAT = 6                    # full patterns (252 channels)
R = 3 * CHP                 # 126 rows per pattern tile
TCH = 4                     # tail channels (252..255)
TB = 8                      # tail position blocks of F
TU = TCH * TB               # 32 tail units
TR = 3 * TU                 # 96 tail rows
F = 1024                    # unit free-dim chunk (PSUM tile = 2 banks f32)
MMN = 512                   # matmul free-dim slice (one PSUM bank)
NCHUNK = POSC // F          # 8
NU = NCHUNK * NPAT          # 48 pattern units; unit NU is the tail
HF = 4 * F                  # xE half-tile columns (4096)
OF = 2 * F                  # osb out-batch columns (2048)

LAST_RESULTS = None         # test.py introspects this


def _softplus(x):
    return np.logaddexp(0.0, x.astype(np.float64))


def _host_params(m0, m1, m2, m3, b0, b1, b2, b3, f0, f1, f2):
    """Fold weights/biases/gates into the device parameterization (float64)."""
    W0 = _softplus(m0)[:, :, 0]
    W1 = _softplus(m1)
    W2 = _softplus(m2)
    W3 = _softplus(m3)[:, 0, :]
    b0_ = b0.astype(np.float64)[:, :, 0]
    b1_ = b1.astype(np.float64)[:, :, 0]
    b2_ = b2.astype(np.float64)[:, :, 0]
    b3_ = b3.astype(np.float64)[:, 0, 0]
    T0 = np.tanh(f0.astype(np.float64))[:, :, 0]
    T1 = np.tanh(f1.astype(np.float64))[:, :, 0]
    T2 = np.tanh(f2.astype(np.float64))[:, :, 0]
    c0 = b0_
    c1 = np.einsum("cjk,ck->cj", W1, b0_) + b1_
    c2 = np.einsum("cjk,ck->cj", W2, c1) + b2_
    c3 = np.einsum("ck,ck->c", W3, c2) + b3_
    A1x = np.einsum("cjk,ck->cj", W1, W0)      # W1 @ W0
    A1t = W1 * T0[:, None, :]                  # W1 diag(T0)
    A3z = np.einsum("cm,cmk->ck", W3, W2)      # W3 @ W2
    A3t = W3 * T2                              # W3 diag(T2)
    return dict(A1x=A1x, A1t=A1t, W2=W2, A3z=A3z, A3t=A3t, W0=W0,
                c0=c0, c1=c1, c2=c2, c3=c3, T1=T1)


def _device_arrays(p):
    """Shared (core-independent) device arrays from host params `p`."""
    f32 = np.float32
    bf16 = ml_dtypes.bfloat16
    arrs = {}
    c = np.arange(CHP)
    for P in range(NPAT):
        sl = slice(CHP * P, CHP * P + CHP)
        A1x, A1t, W2 = p["A1x"][sl], p["A1t"][sl], p["W2"][sl]
        A3z, A3t = p["A3z"][sl], p["A3t"][sl]
        l1t = np.zeros((R, R), f32)
        l2 = np.zeros((R, R), f32)
        d1 = np.zeros((CHP, R), f32)
        g = P % 3
        l3z = np.zeros((R, R), f32)
        l3t = np.zeros((R, R), f32)
        for j in range(3):
            d1[c, CHP * j + c] = A1x[:, j]
            for k in range(3):
                l1t[CHP * k + c, CHP * j + c] = A1t[:, j, k]
                l2[CHP * k + c, CHP * j + c] = W2[:, j, k]
        for k in range(3):
            l3z[CHP * k + c, CHP * g + c] = A3z[:, k]
            l3t[CHP * k + c, CHP * g + c] = A3t[:, k]
        vec = lambda t: np.concatenate(
            [t[:, j] for j in range(3)]).astype(f32).reshape(R, 1)
        arrs[f"l1t{P}"] = l1t
        arrs[f"d1_{P}"] = d1.astype(bf16)
        arrs[f"l2_{P}"] = l2
        arrs[f"l3z{P}"] = l3z
        arrs[f"l3t{P}"] = l3t
        arrs[f"W0v{P}"] = vec(p["W0"][sl])
        arrs[f"c0v{P}"] = vec(p["c0"][sl])
        arrs[f"c1v{P}"] = vec(p["c1"][sl])
        arrs[f"c2v{P}"] = vec(p["c2"][sl])
        arrs[f"T1v{P}"] = vec(p["T1"][sl])
    arrs["c3vA"] = p["c3"][0:126].astype(f32).reshape(126, 1)
    arrs["c3vB"] = p["c3"][126:252].astype(f32).reshape(126, 1)
    # tail: channels 252..255, unit u = 8c + b, rows 32j + u
    u = np.arange(TU)
    chu = 252 + u // TB
    l1tT = np.zeros((TR, TR), f32)
    l2T = np.zeros((TR, TR), f32)
    d1T = np.zeros((TU, TR), f32)
    l3zT = np.zeros((TR, TU), f32)
    l3tT = np.zeros((TR, TU), f32)
    for j in range(3):
        d1T[u, TU * j + u] = p["A1x"][chu, j]
        for k in range(3):
            l1tT[TU * k + u, TU * j + u] = p["A1t"][chu, j, k]
            l2T[TU * k + u, TU * j + u] = p["W2"][chu, j, k]
    for k in range(3):
        l3zT[TU * k + u, u] = p["A3z"][chu, k]
        l3tT[TU * k + u, u] = p["A3t"][chu, k]
    vecT = lambda t: np.concatenate(
        [t[chu, j] for j in range(3)]).astype(f32).reshape(TR, 1)
    arrs.update(l1tT=l1tT, d1T=d1T.astype(bf16), l2T=l2T, l3zT=l3zT,
                l3tT=l3tT,
                W0vT=vecT(p["W0"]), c0vT=vecT(p["c0"]), c1vT=vecT(p["c1"]),
                c2vT=vecT(p["c2"]), T1vT=vecT(p["T1"]),
                c3vT=p["c3"][chu].astype(f32).reshape(TU, 1))
    return arrs


# (name, shape, dtype) of every device parameter
def _param_specs():
    specs = []
    for P in range(NPAT):
        specs += [(f"l1t{P}", [R, R], F32R), (f"d1_{P}", [CHP, R], BF16),
                  (f"l2_{P}", [R, R], F32R), (f"l3z{P}", [R, R], F32R),
                  (f"l3t{P}", [R, R], F32R)]
        specs += [(f"{v}{P}", [R, 1], F32)
                  for v in ("W0v", "c0v", "c1v", "c2v", "T1v")]
    specs += [("c3vA", [126, 1], F32), ("c3vB", [126, 1], F32)]
    specs += [("l1tT", [TR, TR], F32R), ("d1T", [TU, TR], BF16),
              ("l2T", [TR, TR], F32R), ("l3zT", [TR, TU], F32R),
              ("l3tT", [TR, TU], F32R)]
    specs += [(f"{v}T", [TR, 1], F32)
              for v in ("W0v", "c0v", "c1v", "c2v", "T1v")]
    specs += [("c3vT", [TU, 1], F32)]
    return specs


def build_nc(repeat=1, variant="full"):
    """Build the per-core Bass program (SPMD: same program + weights,
    per-core x/o). variant: full | compute_only | dma_only"""
    do_dma = variant != "compute_only"
    do_compute = variant != "dma_only"
    nc = bacc.Bacc("TRN2", target_bir_lowering=False, debug=False)
    x_d = nc.declare_dram_parameter("x", [CH, POSC], F32, isOutput=False)
    o_d = nc.declare_dram_parameter("o", [CH, POSC], F32, isOutput=True)
    pd = {}
    for name, shape, dt in _param_specs():
        pd[name] = (nc.declare_dram_parameter(name, shape, dt, isOutput=False), dt)

    Tanh = mybir.ActivationFunctionType.Tanh
    mult = mybir.AluOpType.mult
    add = mybir.AluOpType.add

    with tile.TileContext(nc) as tc, ExitStack() as ctx:
        singles = ctx.enter_context(tc.tile_pool(name="singles", bufs=1))
        work = ctx.enter_context(tc.tile_pool(name="work", bufs=1))
        psum = ctx.enter_context(tc.tile_pool(name="psum", bufs=1, space="PSUM"))

        w = {}
        for name, (d, dt) in pd.items():
            t = singles.tile(list(d.shape), dt, tag=name, name=name)
            nc.sync.dma_start(out=t[:], in_=d[:])
            w[name] = t

        # resident staging tiles (allocated once; rewritten every repeat)
        xc = [singles.tile([126, POSC], BF16, tag=f"xc{G}", name=f"xc{G}")
              for G in range(2)]
        xe = {(h, P): singles.tile([R, HF], BF16, tag=f"xe{h}_{P}",
                                   name=f"xe{h}_{P}")
              for h in range(2) for P in range(NPAT)}
        xet = singles.tile([TR, F], BF16, tag="xet", name="xet")

        xa = x_d[:]
        oa = o_d[:]

        def emit_xc_loads():
            for G in range(2):
                src = bass.AP(tensor=xa.tensor, offset=xa.offset + 126 * G * POSC,
                              ap=[[POSC, 126], [1, POSC]])
                nc.gpsimd.dma_start(out=xc[G][:], in_=src)

        def emit_reps(h):
            for P in range(NPAT):
                G, g = P // 3, P % 3
                src = xc[G][CHP * g:CHP * g + CHP, HF * h:HF * h + HF]
                for j in range(3):
                    nc.sync.dma_start(
                        out=xe[(h, P)][CHP * j:CHP * j + CHP, 0:HF], in_=src)

        def emit_tail_load():
            src = bass.AP(tensor=xa.tensor, offset=xa.offset + 252 * POSC,
                          ap=[[0, 3], [F, TU], [1, F]])
            nc.gpsimd.dma_start(out=xet[0:TR, 0:F], in_=src)

        def o_dst_ap(t2, g):
            return bass.AP(tensor=oa.tensor,
                           offset=oa.offset + 126 * g * POSC + OF * t2,
                           ap=[[POSC, 126], [1, OF]])

        def o_tail_ap():
            return bass.AP(tensor=oa.tensor, offset=oa.offset + 252 * POSC,
                           ap=[[POSC, TCH], [F, TB], [1, F]])

        def udims(u):
            """(rows, xrows, suffix) for unit u."""
            if u == NU:
                return TR, TU, "T"
            return R, CHP, str(u % NPAT)

        def xe_slice(u, lo, hi, cs0, cs1):
            """AP of unit u's x rows [lo,hi) cols [cs0,cs1) of F-chunk."""
            if u == NU:
                return xet[lo:hi, cs0:cs1]
            t, P = divmod(u, NPAT)
            base = (t % 4) * F
            return xe[(t // 4, P)][lo:hi, base + cs0:base + cs1]

        loop_cm = tc.For_i(0, repeat, 1) if repeat > 1 else nullcontext()

        def bootstrap():
            emit_xc_loads()
            emit_reps(0)
            emit_reps(1)
            emit_tail_load()

        if do_dma:
            bootstrap()

        with loop_cm:
            stB, stC, stD, stE, stG = {}, {}, {}, {}, {}
            packs, osbs = {}, {}
            SS = [slice(s * MMN, (s + 1) * MMN) for s in range(F // MMN)]

            def emit_B(u):
                rows, xr, sx = udims(u)
                t0 = work.tile([R, F], F32R, tag="t0", name="t0", bufs=3)
                nc.scalar.activation(t0[0:rows, :], xe_slice(u, 0, rows, 0, F),
                                     Tanh, bias=w["c0v" + sx][:],
                                     scale=w["W0v" + sx][:])
                stB[u] = t0

            def emit_C(u):
                rows, xr, sx = udims(u)
                t0 = stB.pop(u)
                y1 = psum.tile([R, F], F32, tag="y1", name="y1", bufs=2)
                for ss in SS:
                    nc.tensor.matmul(y1[0:rows, ss], w["l1t" + sx][:],
                                     t0[0:rows, ss], start=True, stop=False)
                for ss in SS:
                    nc.tensor.matmul(y1[0:rows, ss],
                                     w[("d1_" + sx) if u != NU else "d1T"][:],
                                     xe_slice(u, 0, xr, ss.start, ss.stop),
                                     start=False, stop=True)
                stC[u] = y1

            def emit_D(u):
                rows, _, sx = udims(u)
                y1 = stC[u]
                t1 = work.tile([R, F], F32R, tag="t1", name="t1", bufs=2)
                nc.scalar.activation(t1[0:rows, :], y1[0:rows, :], Tanh,
                                     bias=w["c1v" + sx][:])
                stD[u] = t1

            def emit_E(u):
                rows, _, sx = udims(u)
                y1 = stC.pop(u)
                t1 = stD.pop(u)
                z2 = work.tile([R, F], F32R, tag="z2", name="z2", bufs=2)
                nc.vector.scalar_tensor_tensor(
                    z2[0:rows, :], t1[0:rows, :], w["T1v" + sx][:],
                    y1[0:rows, :], op0=mult, op1=add)
                stE[u] = z2

            def emit_F(u):
                rows, _, sx = udims(u)
                z2 = stE[u]
                y2 = psum.tile([R, F], F32, tag="y2", name="y2", bufs=1)
                for ss in SS:
                    nc.tensor.matmul(y2[0:rows, ss],
                                     w[("l2_" + sx) if u != NU else "l2T"][:],
                                     z2[0:rows, ss], start=True, stop=True)
                stG[u] = y2

            def emit_G(u):
                rows, _, sx = udims(u)
                y2 = stG.pop(u)
                t2 = work.tile([R, F], F32R, tag="t2", name="t2", bufs=2)
                nc.scalar.activation(t2[0:rows, :], y2[0:rows, :], Tanh,
                                     bias=w["c2v" + sx][:])
                stD[("t2", u)] = t2

            def emit_H(u):
                rows, _, sx = udims(u)
                z2 = stE.pop(u)
                t2 = stD.pop(("t2", u))
                if u == NU:
                    pk = psum.tile([R, F], F32, tag="pack", name="pack", bufs=1)
                    for ss in SS:
                        nc.tensor.matmul(pk[0:TU, ss], w["l3zT"][:],
                                         z2[0:TR, ss], start=True, stop=False)
                    for ss in SS:
                        nc.tensor.matmul(pk[0:TU, ss], w["l3tT"][:],
                                         t2[0:TR, ss], start=False, stop=True)
                    osbt = work.tile([TU, F], F32, tag="osbt", name="osbt",
                                     bufs=2)
                    nc.vector.tensor_scalar_add(osbt[0:TU, :], pk[0:TU, :],
                                                w["c3vT"][:])
                    if do_dma:
                        nc.gpsimd.dma_start(out=o_tail_ap(),
                                            in_=osbt[0:TU, 0:F])
                    return
                t, P = divmod(u, NPAT)
                g = P // 3
                first, last = P % 3 == 0, P % 3 == 2
                if first:
                    packs[(t, g)] = psum.tile([R, F], F32, tag="pack",
                                              name="pack", bufs=1)
                pk = packs[(t, g)]
                for ss in SS:
                    nc.tensor.matmul(pk[:, ss], w["l3z" + sx][:], z2[:, ss],
                                     start=first, stop=False)
                for ss in SS:
                    nc.tensor.matmul(pk[:, ss], w["l3t" + sx][:], t2[:, ss],
                                     start=False, stop=last)
                if last:
                    pk = packs.pop((t, g))
                    t2i, cs = divmod(t, 2)
                    if (t2i, g) not in osbs:
                        osbs[(t2i, g)] = work.tile([R, OF], F32, tag="osb",
                                                   name="osb", bufs=2)
                    osb = osbs[(t2i, g)]
                    nc.vector.tensor_scalar_add(
                        osb[:, cs * F:cs * F + F], pk[:],
                        w["c3vA" if g == 0 else "c3vB"][:])
                    if cs == 1:
                        osb = osbs.pop((t2i, g))
                        if do_dma:
                            nc.gpsimd.dma_start(out=o_dst_ap(t2i, g),
                                                in_=osb[:])

            if do_dma:
                emit_xc_loads()

            if do_compute:
                # prologue
                for u in (0, 1, 2):
                    emit_B(u)
                emit_C(0)
                emit_D(0)
                emit_C(1)
                emit_D(1)
                emit_E(0)
                emit_F(0)
                emit_G(0)
                # steady state
                for i in range(NU + 1):
                    if i + 3 <= NU:
                        emit_B(i + 3)
                    if i + 2 <= NU:
                        emit_C(i + 2)
                        emit_D(i + 2)
                    if i + 1 <= NU:
                        emit_E(i + 1)
                        emit_F(i + 1)
                        emit_G(i + 1)
                    emit_H(i)
                    if i == 23 and do_dma:
                        emit_reps(0)   # xE half 0 reload for next repeat
                if do_dma:
                    emit_reps(1)
                    emit_tail_load()
            else:
                emit_reps(0)
                emit_reps(1)
                emit_tail_load()

    nc.finalize()
    return nc


def kernel(inputs, m0, m1, m2, m3, b0, b1, b2, b3, f0, f1, f2, stop_gradient):
    global LAST_RESULTS
    del stop_gradient  # False in setup_inputs; forward math identical anyway
    in_maps = make_in_maps(inputs, m0, m1, m2, m3, b0, b1, b2, b3, f0, f1, f2)

    nc = build_nc()
    res = run_bass_kernel_spmd(
        nc, in_maps, list(range(NCORES)),
        trace=bool(os.environ.get("BASS_TRACE")))
    LAST_RESULTS = res
    out = np.empty((CH, NPOS), dtype=np.float32)
    for g in range(NCORES):
        out[:, g * POSC:(g + 1) * POSC] = res.results[g]["o"]
    return out.reshape(CH, 1, NPOS)


def measure_exec_ns(in_maps, r1=8, r2=1032, n_wall=3):
    """Device-exec-time proxy: wall-clock delta between repeat=r2 and
    repeat=r1 kernels (upload/dispatch overheads cancel in the delta)."""
    import time as _time
    walls = {}
    for rep in (r1, r2):
        nc = build_nc(repeat=rep)
        best = None
        for it in range(n_wall):
            t0 = _time.perf_counter()
            run_bass_kernel_spmd(nc, in_maps, list(range(NCORES)))
            dt = _time.perf_counter() - t0
            if it > 0:  # first call pays compile
                best = dt if best is None else min(best, dt)
        walls[rep] = best
    return (walls[r2] - walls[r1]) / (r2 - r1) * 1e9, walls


def make_in_maps(inputs, m0, m1, m2, m3, b0, b1, b2, b3, f0, f1, f2):
    inputs = np.asarray(inputs, dtype=np.float32)
    params = _host_params(
        *(np.asarray(a) for a in (m0, m1, m2, m3, b0, b1, b2, b3, f0, f1, f2)))
    arrs = _device_arrays(params)
    x = inputs.reshape(CH, NPOS)
    in_maps = []
    for g in range(NCORES):
        im = {"x": np.ascontiguousarray(x[:, g * POSC:(g + 1) * POSC])}
        im.update(arrs)
        in_maps.append(im)
    return in_maps
